# revision 2
# baseline (speedup 1.0000x reference)
"""Quantized-weight batched linear: out[b,n,m] = sum_k deq(qweight)[n,k] * x[b,k,m].

Strategy:
  - Host: dequantize weight (fp32, exact same formula as the oracle), transpose
    to (K, N), round weights + activations to bf16.
  - Device (8 cores, data-parallel over batch B=64 -> 8 batches/core):
    PE bf16 matmuls, K accumulated in PSUM over 8 chunks of 128,
    N tiled 8x128 (PSUM partitions), M tiled 2x512 (PSUM bank free-dim).
  - Pipeline details tuned from the NTFF trace:
      * x loads on the sync HWDGE queue, w loads + output stores on the
        scalar HWDGE queue (w loads finish before the first store).
      * a few warmup matmuls on a zeroed scratch tile run during the
        initial DMA window so the PE p-state ramp is spent before real work.
      * PSUM groups of 2 n-tiles x 2 m-banks (4 banks live, 8-bank pool)
        so group drains overlap the next group's matmuls.
      * last batch tapers groups [2,2,2,1,1]; the final group runs its two
        m-banks serially and drains the last bank via split copies
        (vector+scalar) and split stores (sync+scalar).
  - Gather core outputs along batch -> (64, 1024, 1024) fp32.
"""

import numpy as np
import ml_dtypes

N = 1024  # output rows (weight rows)
K = 1024  # reduction dim
M = 1024  # columns of x per batch
NGROUP = 16
GS = K // NGROUP
B = 64
NCORES = 8
BPC = B // NCORES  # batches per core

NWARM = 4  # PE warmup matmuls on scratch data

_CACHE = {}
LAST_RESULT = None  # BassKernelResults of the most recent run (for profiling)


def _build_nc(bpc=BPC, k=K, n=N, m=M, nwarm=NWARM):
    import concourse.mybir as mybir
    import concourse.tile as tile
    from concourse import bacc

    kc = k // 128   # contraction chunks (partition dim)
    nt = n // 128   # output-row tiles (PSUM partition dim)
    mt = m // 512   # moving free-dim tiles (one PSUM bank each)

    nc = bacc.Bacc(None, target_bir_lowering=False, debug=False)
    wt = nc.dram_tensor("wt", [k, n], mybir.dt.bfloat16, kind="ExternalInput")
    xs = nc.dram_tensor("xs", [bpc, k, m], mybir.dt.bfloat16, kind="ExternalInput")
    out = nc.dram_tensor("out", [bpc, n, m], mybir.dt.float32, kind="ExternalOutput")

    npg = 2  # n-tiles per PSUM group (4 banks live; 8-bank pool double-buffers)

    with tile.TileContext(nc) as tc:
        with (
            tc.tile_pool(name="wpool", bufs=1) as wpool,
            tc.tile_pool(name="xpool", bufs=2 * kc * mt) as xpool,
            tc.tile_pool(name="opool", bufs=8) as opool,
            tc.tile_pool(name="spool", bufs=1) as spool,
            tc.tile_pool(name="psum", bufs=8, space="PSUM") as psum_pool,
        ):
            # Warmup: a zeroed scratch tile feeds a few matmuls that run
            # while the first DMAs are in flight, absorbing the PE p-state
            # ramp (first ~6 matmuls after idle run 2-4x slow).
            scr = spool.tile([128, 512], mybir.dt.bfloat16, tag="scr", name="scr")
            nc.vector.memset(scr[:], 0.0)
            pwarm = psum_pool.tile([128, 512], mybir.dt.float32, tag="ps", name="pswarm")
            for i in range(nwarm):
                nc.tensor.matmul(pwarm[:], scr[:, 0:128], scr[:], start=True, stop=True)

            # x loads (sync queue): [128, 512] tiles keyed (batch, kchunk, mhalf).
            # w loads (scalar queue): chunk 0 split so the first LDWEIGHTS only
            # waits on a 64KB transfer.
            xtiles = {}  # (b, kk, m0) -> tile

            def load_x(b):
                for kk in range(kc):
                    for m0 in range(mt):
                        t = xpool.tile([128, 512], mybir.dt.bfloat16, tag="x",
                                       name=f"x{b}_{kk}_{m0}")
                        nc.sync.dma_start(
                            out=t[:],
                            in_=xs[b, kk * 128:(kk + 1) * 128, m0 * 512:(m0 + 1) * 512],
                        )
                        xtiles[b, kk, m0] = t

            w0a = wpool.tile([128, 256], mybir.dt.bfloat16, tag="w0a", name="w0a")
            nc.scalar.dma_start(out=w0a[:], in_=wt[0:128, 0:256])
            # first x tile after the first w piece on its own queue
            load_x(0)
            w0b = wpool.tile([128, 768], mybir.dt.bfloat16, tag="w0b", name="w0b")
            nc.scalar.dma_start(out=w0b[:], in_=wt[0:128, 256:1024])
            wsb = {}
            for kk in range(1, kc):
                wtile = wpool.tile([128, n], mybir.dt.bfloat16, tag=f"w{kk}", name=f"w{kk}")
                nc.scalar.dma_start(out=wtile[:], in_=wt[kk * 128:(kk + 1) * 128, :])
                wsb[kk] = wtile

            def w_slice(kk, n0):
                if kk == 0:
                    if n0 < 2:
                        return w0a[:, n0 * 128:(n0 + 1) * 128]
                    return w0b[:, (n0 - 2) * 128:(n0 - 1) * 128]
                return wsb[kk][:, n0 * 128:(n0 + 1) * 128]

            for b in range(bpc):
                if b + 1 < bpc:
                    load_x(b + 1)

                groups = [2, 2, 2, 1, 1] if b == bpc - 1 else [npg] * (nt // npg)
                n0_base = 0
                for h, gsz in enumerate(groups):
                    final = b == bpc - 1 and h == len(groups) - 1
                    ps = {}
                    for j in range(gsz):
                        for m0 in range(mt):
                            ps[j, m0] = psum_pool.tile(
                                [128, 512], mybir.dt.float32, tag="ps",
                                name=f"ps{b}_{h}_{j}_{m0}"
                            )
                    if final:
                        # Serialize the two m-banks so the first drains while
                        # the second still computes; split the very last
                        # drain across engines/queues.
                        n0 = n0_base
                        for m0 in range(mt):
                            for kk in range(kc):
                                nc.tensor.matmul(
                                    ps[0, m0][:], w_slice(kk, n0),
                                    xtiles[b, kk, m0][:],
                                    start=(kk == 0), stop=(kk == kc - 1),
                                )
                            ot = opool.tile([128, 512], mybir.dt.float32, tag="o",
                                            name=f"o{b}_{n0}_{m0}")
                            dst = out[b, n0 * 128:(n0 + 1) * 128,
                                      m0 * 512:(m0 + 1) * 512]
                            if m0 == 0:
                                nc.vector.tensor_copy(ot[:], ps[0, m0][:])
                                nc.scalar.dma_start(out=dst, in_=ot[:])
                            else:
                                nc.vector.tensor_copy(ot[:, 0:256], ps[0, m0][:, 0:256])
                                nc.scalar.copy(ot[:, 256:512], ps[0, m0][:, 256:512])
                                nc.sync.dma_start(
                                    out=out[b, n0 * 128:(n0 + 1) * 128,
                                            m0 * 512 + 0:m0 * 512 + 256],
                                    in_=ot[:, 0:256],
                                )
                                nc.scalar.dma_start(
                                    out=out[b, n0 * 128:(n0 + 1) * 128,
                                            m0 * 512 + 256:m0 * 512 + 512],
                                    in_=ot[:, 256:512],
                                )
                    else:
                        # k-outer accumulation into gsz*mt PSUM banks: every x
                        # chunk is fully consumed on arrival.
                        for kk in range(kc):
                            for j in range(gsz):
                                n0 = n0_base + j
                                lhsT = w_slice(kk, n0)
                                for m0 in range(mt):
                                    nc.tensor.matmul(
                                        ps[j, m0][:], lhsT,
                                        xtiles[b, kk, m0][:],
                                        start=(kk == 0), stop=(kk == kc - 1),
                                    )
                        for j in range(gsz):
                            n0 = n0_base + j
                            for m0 in range(mt):
                                ot = opool.tile([128, 512], mybir.dt.float32, tag="o",
                                                name=f"o{b}_{n0}_{m0}")
                                nc.vector.tensor_copy(ot[:], ps[j, m0][:])
                                nc.scalar.dma_start(
                                    out=out[b, n0 * 128:(n0 + 1) * 128,
                                            m0 * 512:(m0 + 1) * 512],
                                    in_=ot[:],
                                )
                    n0_base += gsz
    nc.compile()
    return nc


def _dequant_wt(qweight, qrange, qmin):
    # Matches reference: w = q * qrange + qmin per (row, group), fp32.
    q = np.asarray(qweight).astype(np.float32).reshape(N, NGROUP, GS)
    qr = np.asarray(qrange).astype(np.float32).reshape(N, NGROUP, 1)
    qm = np.asarray(qmin).astype(np.float32).reshape(N, NGROUP, 1)
    w = (q * qr + qm).reshape(N, K)
    return np.ascontiguousarray(w.T).astype(ml_dtypes.bfloat16)  # (K, N)


def _ensure_axon_hooks():
    """run_bass_kernel_spmd(trace=True) imports antenv.axon_hooks, which some
    images lack; provide a stub (and register the real NTFF hook if the boot
    package is present) so tracing degrades gracefully instead of crashing."""
    try:
        import antenv.axon_hooks  # noqa: F401
        return
    except ImportError:
        pass
    try:
        import sys
        import types

        import antenv

        mod = types.ModuleType("antenv.axon_hooks")
        mod._hook = None
        mod.set_axon_ntff_profile_hook = lambda h: setattr(mod, "_hook", h)
        mod.get_axon_ntff_profile_hook = lambda: mod._hook
        sys.modules["antenv.axon_hooks"] = mod
        antenv.axon_hooks = mod
        try:
            from trn_agent_boot.trn_boot import _ntff_profile_via_ctypes

            mod._hook = _ntff_profile_via_ctypes("/opt/axon/libaxon_pjrt.so")
        except Exception:
            pass
    except Exception:
        pass


def kernel(x, qweight, qrange, qmin):
    global LAST_RESULT
    _ensure_axon_hooks()
    from concourse.bass_utils import run_bass_kernel_spmd

    wt_host = _dequant_wt(qweight, qrange, qmin)
    xb = np.asarray(x).astype(ml_dtypes.bfloat16)  # (B, K, M)

    if "nc" not in _CACHE:
        _CACHE["nc"] = _build_nc()
    nc = _CACHE["nc"]

    in_maps = [
        {"wt": wt_host, "xs": np.ascontiguousarray(xb[c * BPC:(c + 1) * BPC])}
        for c in range(NCORES)
    ]
    LAST_RESULT = run_bass_kernel_spmd(nc, in_maps, core_ids=list(range(NCORES)))
    outs = [r["out"] for r in LAST_RESULT.results]
    return np.ascontiguousarray(np.concatenate(outs, axis=0)).astype(np.float32, copy=False)


# revision 6
# speedup vs baseline: 1.0193x; 1.0193x over previous
"""Quantized-weight batched linear: out[b,n,m] = sum_k deq(qweight)[n,k] * x[b,k,m].

Strategy:
  - Host: dequantize weight (fp32, exact same formula as the oracle), transpose
    to (K, N), round weights + activations to bf16.
  - Device (8 cores, data-parallel over batch B=64 -> 8 batches/core):
    PE bf16 matmuls, K accumulated in PSUM over 8 chunks of 128,
    N tiled 8x128 (PSUM partitions), M tiled 2x512 (PSUM bank free-dim).
  - Pipeline details tuned from the NTFF trace:
      * x loads on the sync HWDGE queue, w loads + output stores on the
        scalar HWDGE queue (w loads finish before the first store).
      * a few warmup matmuls on a zeroed scratch tile run during the
        initial DMA window so the PE p-state ramp is spent before real work.
      * PSUM groups of 2 n-tiles x 2 m-banks (4 banks live, 8-bank pool)
        so group drains overlap the next group's matmuls.
      * last batch tapers groups [2,2,2,1,1]; the final group runs its two
        m-banks serially and drains the last bank via split copies
        (vector+scalar) and split stores (sync+scalar).
  - Gather core outputs along batch -> (64, 1024, 1024) fp32.
"""

import numpy as np
import ml_dtypes

N = 1024  # output rows (weight rows)
K = 1024  # reduction dim
M = 1024  # columns of x per batch
NGROUP = 16
GS = K // NGROUP
B = 64
NCORES = 8
BPC = B // NCORES  # batches per core

NWARM = 3  # PE warmup matmuls on scratch data

_CACHE = {}
LAST_RESULT = None  # BassKernelResults of the most recent run (for profiling)


def _build_nc(bpc=BPC, k=K, n=N, m=M, nwarm=NWARM):
    import concourse.mybir as mybir
    import concourse.tile as tile
    from concourse import bacc

    kc = k // 128   # contraction chunks (partition dim)
    nt = n // 128   # output-row tiles (PSUM partition dim)
    mt = m // 512   # moving free-dim tiles (one PSUM bank each)

    nc = bacc.Bacc(None, target_bir_lowering=False, debug=False)
    wt = nc.dram_tensor("wt", [k, n], mybir.dt.bfloat16, kind="ExternalInput")
    xs = nc.dram_tensor("xs", [bpc, k, m], mybir.dt.bfloat16, kind="ExternalInput")
    out = nc.dram_tensor("out", [bpc, n, m], mybir.dt.float32, kind="ExternalOutput")

    npg = 2  # n-tiles per PSUM group (4 banks live; 8-bank pool double-buffers)

    with tile.TileContext(nc) as tc:
        with (
            tc.tile_pool(name="wpool", bufs=1) as wpool,
            tc.tile_pool(name="xpool", bufs=2 * kc * mt) as xpool,
            tc.tile_pool(name="opool", bufs=8) as opool,
            tc.tile_pool(name="spool", bufs=1) as spool,
            tc.tile_pool(name="psum", bufs=8, space="PSUM") as psum_pool,
        ):
            # Warmup: a zeroed scratch tile feeds a few matmuls that run
            # while the first DMAs are in flight, absorbing the PE p-state
            # ramp (first ~6 matmuls after idle run 2-4x slow).
            scr = spool.tile([128, 512], mybir.dt.bfloat16, tag="scr", name="scr")
            nc.vector.memset(scr[:], 0.0)
            pwarm = psum_pool.tile([128, 512], mybir.dt.float32, tag="ps", name="pswarm")
            for i in range(nwarm):
                nc.tensor.matmul(pwarm[:], scr[:, 0:128], scr[:], start=True, stop=True)

            # x loads (sync queue): [128, 512] tiles keyed (batch, kchunk, mhalf).
            # w loads (scalar queue): chunk 0 split so the first LDWEIGHTS only
            # waits on a 64KB transfer.
            xtiles = {}  # (b, kk, m0) -> tile

            def load_x(b):
                for kk in range(kc):
                    for m0 in range(mt):
                        t = xpool.tile([128, 512], mybir.dt.bfloat16, tag="x",
                                       name=f"x{b}_{kk}_{m0}")
                        nc.sync.dma_start(
                            out=t[:],
                            in_=xs[b, kk * 128:(kk + 1) * 128, m0 * 512:(m0 + 1) * 512],
                        )
                        xtiles[b, kk, m0] = t

            w0a = wpool.tile([128, 256], mybir.dt.bfloat16, tag="w0a", name="w0a")
            nc.scalar.dma_start(out=w0a[:], in_=wt[0:128, 0:256])
            # first x tile after the first w piece on its own queue
            load_x(0)
            w0b = wpool.tile([128, 768], mybir.dt.bfloat16, tag="w0b", name="w0b")
            nc.scalar.dma_start(out=w0b[:], in_=wt[0:128, 256:1024])
            wsb = {}
            for kk in range(1, kc):
                wtile = wpool.tile([128, n], mybir.dt.bfloat16, tag=f"w{kk}", name=f"w{kk}")
                nc.scalar.dma_start(out=wtile[:], in_=wt[kk * 128:(kk + 1) * 128, :])
                wsb[kk] = wtile

            def w_slice(kk, n0):
                if kk == 0:
                    if n0 < 2:
                        return w0a[:, n0 * 128:(n0 + 1) * 128]
                    return w0b[:, (n0 - 2) * 128:(n0 - 1) * 128]
                return wsb[kk][:, n0 * 128:(n0 + 1) * 128]

            for b in range(bpc):
                if b + 1 < bpc:
                    load_x(b + 1)

                if b == 0:
                    # Wide first group: consumes x at half rate so the DMA
                    # stream builds a lead instead of racing the PE.
                    groups = [4, 2, 2]
                elif b == bpc - 1:
                    groups = [2, 2, 2, 1, 1]
                else:
                    groups = [npg] * (nt // npg)
                n0_base = 0
                for h, gsz in enumerate(groups):
                    final = b == bpc - 1 and h == len(groups) - 1
                    if final:
                        # Final n-tile: three m-pieces [512, 256, 256] run
                        # serially so earlier pieces drain (and absorb DMA
                        # doorbell latency) under the later pieces' matmuls.
                        n0 = n0_base
                        pieces = [(0, 512), (512, 256), (768, 256)]
                        for pi, (moff, mw) in enumerate(pieces):
                            # full-bank PSUM tile; only the first mw columns
                            # are written (keeps bank-granular allocation)
                            pbank = psum_pool.tile(
                                [128, 512], mybir.dt.float32, tag="ps",
                                name=f"psf_{pi}"
                            )
                            pst = pbank[:, 0:mw]
                            for kk in range(kc):
                                m0 = moff // 512
                                sub = slice(moff - m0 * 512, moff - m0 * 512 + mw)
                                nc.tensor.matmul(
                                    pst, w_slice(kk, n0),
                                    xtiles[b, kk, m0][:, sub],
                                    start=(kk == 0), stop=(kk == kc - 1),
                                )
                            ot = opool.tile([128, mw], mybir.dt.float32, tag="o",
                                            name=f"of_{pi}")
                            dst = out[b, n0 * 128:(n0 + 1) * 128, moff:moff + mw]
                            nc.vector.tensor_copy(ot[:], pst)
                            if pi < 2:
                                nc.scalar.dma_start(out=dst, in_=ot[:])
                            else:
                                nc.sync.dma_start(out=dst, in_=ot[:])
                        n0_base += gsz
                        continue
                    ps = {}
                    for j in range(gsz):
                        for m0 in range(mt):
                            ps[j, m0] = psum_pool.tile(
                                [128, 512], mybir.dt.float32, tag="ps",
                                name=f"ps{b}_{h}_{j}_{m0}"
                            )
                    # k-outer accumulation into gsz*mt PSUM banks: every x
                    # chunk is fully consumed on arrival.
                    for kk in range(kc):
                        if b == 0 and h == 0 and kk == 0:
                            # m-outer for the very first chunk: the first 4
                            # matmuls only need the (0,0,0) tile while the
                            # (0,0,1) tile is still in flight.
                            order = [(j, m0) for m0 in range(mt) for j in range(gsz)]
                        else:
                            order = [(j, m0) for j in range(gsz) for m0 in range(mt)]
                        for j, m0 in order:
                            n0 = n0_base + j
                            nc.tensor.matmul(
                                ps[j, m0][:], w_slice(kk, n0),
                                xtiles[b, kk, m0][:],
                                start=(kk == 0), stop=(kk == kc - 1),
                            )
                    for j in range(gsz):
                        n0 = n0_base + j
                        for m0 in range(mt):
                            ot = opool.tile([128, 512], mybir.dt.float32, tag="o",
                                            name=f"o{b}_{n0}_{m0}")
                            nc.vector.tensor_copy(ot[:], ps[j, m0][:])
                            nc.scalar.dma_start(
                                out=out[b, n0 * 128:(n0 + 1) * 128,
                                        m0 * 512:(m0 + 1) * 512],
                                in_=ot[:],
                            )
                    n0_base += gsz
    nc.compile()
    return nc


def _dequant_wt(qweight, qrange, qmin):
    # Matches reference: w = q * qrange + qmin per (row, group), fp32.
    q = np.asarray(qweight).astype(np.float32).reshape(N, NGROUP, GS)
    qr = np.asarray(qrange).astype(np.float32).reshape(N, NGROUP, 1)
    qm = np.asarray(qmin).astype(np.float32).reshape(N, NGROUP, 1)
    w = (q * qr + qm).reshape(N, K)
    return np.ascontiguousarray(w.T).astype(ml_dtypes.bfloat16)  # (K, N)


def _ensure_axon_hooks():
    """run_bass_kernel_spmd(trace=True) imports antenv.axon_hooks, which some
    images lack; provide a stub (and register the real NTFF hook if the boot
    package is present) so tracing degrades gracefully instead of crashing."""
    try:
        import antenv.axon_hooks  # noqa: F401
        return
    except ImportError:
        pass
    try:
        import sys
        import types

        import antenv

        mod = types.ModuleType("antenv.axon_hooks")
        mod._hook = None
        mod.set_axon_ntff_profile_hook = lambda h: setattr(mod, "_hook", h)
        mod.get_axon_ntff_profile_hook = lambda: mod._hook
        sys.modules["antenv.axon_hooks"] = mod
        antenv.axon_hooks = mod
        try:
            from trn_agent_boot.trn_boot import _ntff_profile_via_ctypes

            mod._hook = _ntff_profile_via_ctypes("/opt/axon/libaxon_pjrt.so")
        except Exception:
            pass
    except Exception:
        pass


def kernel(x, qweight, qrange, qmin):
    global LAST_RESULT
    _ensure_axon_hooks()
    from concourse.bass_utils import run_bass_kernel_spmd

    wt_host = _dequant_wt(qweight, qrange, qmin)
    xb = np.asarray(x).astype(ml_dtypes.bfloat16)  # (B, K, M)

    if "nc" not in _CACHE:
        _CACHE["nc"] = _build_nc()
    nc = _CACHE["nc"]

    in_maps = [
        {"wt": wt_host, "xs": np.ascontiguousarray(xb[c * BPC:(c + 1) * BPC])}
        for c in range(NCORES)
    ]
    LAST_RESULT = run_bass_kernel_spmd(nc, in_maps, core_ids=list(range(NCORES)))
    outs = [r["out"] for r in LAST_RESULT.results]
    return np.ascontiguousarray(np.concatenate(outs, axis=0)).astype(np.float32, copy=False)


# revision 8
# speedup vs baseline: 1.0214x; 1.0020x over previous
"""Quantized-weight batched linear: out[b,n,m] = sum_k deq(qweight)[n,k] * x[b,k,m].

Strategy:
  - Host: dequantize weight (fp32, exact same formula as the oracle), transpose
    to (K, N), round weights + activations to bf16.
  - Device (8 cores, data-parallel over batch B=64 -> 8 batches/core):
    PE bf16 matmuls, K accumulated in PSUM over 8 chunks of 128,
    N tiled 8x128 (PSUM partitions), M tiled 2x512 (PSUM bank free-dim).
  - Pipeline details tuned from the NTFF trace:
      * x loads on the sync HWDGE queue, w loads + output stores on the
        scalar HWDGE queue (w loads finish before the first store).
      * a few warmup matmuls on a zeroed scratch tile run during the
        initial DMA window so the PE p-state ramp is spent before real work.
      * PSUM groups of 2 n-tiles x 2 m-banks (4 banks live, 8-bank pool)
        so group drains overlap the next group's matmuls.
      * last batch tapers groups [2,2,2,1,1]; the final group runs its two
        m-banks serially and drains the last bank via split copies
        (vector+scalar) and split stores (sync+scalar).
  - Gather core outputs along batch -> (64, 1024, 1024) fp32.
"""

import numpy as np
import ml_dtypes

N = 1024  # output rows (weight rows)
K = 1024  # reduction dim
M = 1024  # columns of x per batch
NGROUP = 16
GS = K // NGROUP
B = 64
NCORES = 8
BPC = B // NCORES  # batches per core

NWARM = 8  # PE warmup matmuls on scratch data

_CACHE = {}
LAST_RESULT = None  # BassKernelResults of the most recent run (for profiling)


def _build_nc(bpc=BPC, k=K, n=N, m=M, nwarm=NWARM):
    import concourse.mybir as mybir
    import concourse.tile as tile
    from concourse import bacc

    kc = k // 128   # contraction chunks (partition dim)
    nt = n // 128   # output-row tiles (PSUM partition dim)
    mt = m // 512   # moving free-dim tiles (one PSUM bank each)

    nc = bacc.Bacc(None, target_bir_lowering=False, debug=False)
    wt = nc.dram_tensor("wt", [k, n], mybir.dt.bfloat16, kind="ExternalInput")
    xs = nc.dram_tensor("xs", [bpc, k, m], mybir.dt.bfloat16, kind="ExternalInput")
    out = nc.dram_tensor("out", [bpc, n, m], mybir.dt.float32, kind="ExternalOutput")

    npg = 2  # n-tiles per PSUM group (4 banks live; 8-bank pool double-buffers)

    with tile.TileContext(nc) as tc:
        with (
            tc.tile_pool(name="wpool", bufs=1) as wpool,
            tc.tile_pool(name="xpool", bufs=2 * kc * mt) as xpool,
            tc.tile_pool(name="opool", bufs=8) as opool,
            tc.tile_pool(name="spool", bufs=1) as spool,
            tc.tile_pool(name="psum", bufs=8, space="PSUM") as psum_pool,
        ):
            # Warmup: a zeroed scratch tile feeds a few matmuls that run
            # while the first DMAs are in flight, absorbing the PE p-state
            # ramp (first ~6 matmuls after idle run 2-4x slow).
            scr = spool.tile([128, 512], mybir.dt.bfloat16, tag="scr", name="scr")
            nc.gpsimd.memset(scr[:], 0.0)
            pwarm = psum_pool.tile([128, 512], mybir.dt.float32, tag="ps", name="pswarm")
            for i in range(nwarm):
                nc.tensor.matmul(pwarm[:], scr[:, 0:128], scr[:], start=True, stop=True)

            # x loads (sync queue): [128, 512] tiles keyed (batch, kchunk, mhalf).
            # w loads (scalar queue): chunk 0 split so the first LDWEIGHTS only
            # waits on a 64KB transfer.
            xtiles = {}  # (b, kk, m0) -> tile

            def load_x(b):
                for kk in range(kc):
                    for m0 in range(mt):
                        t = xpool.tile([128, 512], mybir.dt.bfloat16, tag="x",
                                       name=f"x{b}_{kk}_{m0}")
                        nc.sync.dma_start(
                            out=t[:],
                            in_=xs[b, kk * 128:(kk + 1) * 128, m0 * 512:(m0 + 1) * 512],
                        )
                        xtiles[b, kk, m0] = t

            w0a = wpool.tile([128, 256], mybir.dt.bfloat16, tag="w0a", name="w0a")
            nc.scalar.dma_start(out=w0a[:], in_=wt[0:128, 0:256])
            # first x tile after the first w piece on its own queue
            load_x(0)
            w0b = wpool.tile([128, 768], mybir.dt.bfloat16, tag="w0b", name="w0b")
            nc.scalar.dma_start(out=w0b[:], in_=wt[0:128, 256:1024])
            wsb = {}
            for kk in range(1, kc):
                wtile = wpool.tile([128, n], mybir.dt.bfloat16, tag=f"w{kk}", name=f"w{kk}")
                nc.scalar.dma_start(out=wtile[:], in_=wt[kk * 128:(kk + 1) * 128, :])
                wsb[kk] = wtile

            def w_slice(kk, n0):
                if kk == 0:
                    if n0 < 2:
                        return w0a[:, n0 * 128:(n0 + 1) * 128]
                    return w0b[:, (n0 - 2) * 128:(n0 - 1) * 128]
                return wsb[kk][:, n0 * 128:(n0 + 1) * 128]

            for b in range(bpc):
                if b + 1 < bpc:
                    load_x(b + 1)

                if b == 0:
                    # Wide first group: consumes x at half rate so the DMA
                    # stream builds a lead instead of racing the PE.
                    groups = [4, 2, 2]
                elif b == bpc - 1:
                    groups = [2, 2, 2, 1, 1]
                else:
                    groups = [npg] * (nt // npg)
                n0_base = 0
                for h, gsz in enumerate(groups):
                    final = b == bpc - 1 and h == len(groups) - 1
                    if final:
                        # Final n-tile: three m-pieces [512, 256, 256] run
                        # serially so earlier pieces drain (and absorb DMA
                        # doorbell latency) under the later pieces' matmuls.
                        n0 = n0_base
                        pieces = [(0, 512), (512, 256), (768, 256)]
                        for pi, (moff, mw) in enumerate(pieces):
                            # full-bank PSUM tile; only the first mw columns
                            # are written (keeps bank-granular allocation)
                            pbank = psum_pool.tile(
                                [128, 512], mybir.dt.float32, tag="ps",
                                name=f"psf_{pi}"
                            )
                            pst = pbank[:, 0:mw]
                            for kk in range(kc):
                                m0 = moff // 512
                                sub = slice(moff - m0 * 512, moff - m0 * 512 + mw)
                                nc.tensor.matmul(
                                    pst, w_slice(kk, n0),
                                    xtiles[b, kk, m0][:, sub],
                                    start=(kk == 0), stop=(kk == kc - 1),
                                )
                            ot = opool.tile([128, mw], mybir.dt.float32, tag="o",
                                            name=f"of_{pi}")
                            dst = out[b, n0 * 128:(n0 + 1) * 128, moff:moff + mw]
                            nc.vector.tensor_copy(ot[:], pst)
                            if pi < 2:
                                nc.scalar.dma_start(out=dst, in_=ot[:])
                            else:
                                nc.sync.dma_start(out=dst, in_=ot[:])
                        n0_base += gsz
                        continue
                    ps = {}
                    for j in range(gsz):
                        for m0 in range(mt):
                            ps[j, m0] = psum_pool.tile(
                                [128, 512], mybir.dt.float32, tag="ps",
                                name=f"ps{b}_{h}_{j}_{m0}"
                            )
                    # k-outer accumulation into gsz*mt PSUM banks: every x
                    # chunk is fully consumed on arrival.
                    for kk in range(kc):
                        if b == 0 and h == 0 and kk == 0:
                            # m-outer for the very first chunk: the first 4
                            # matmuls only need the (0,0,0) tile while the
                            # (0,0,1) tile is still in flight.
                            order = [(j, m0) for m0 in range(mt) for j in range(gsz)]
                        else:
                            order = [(j, m0) for j in range(gsz) for m0 in range(mt)]
                        for j, m0 in order:
                            n0 = n0_base + j
                            nc.tensor.matmul(
                                ps[j, m0][:], w_slice(kk, n0),
                                xtiles[b, kk, m0][:],
                                start=(kk == 0), stop=(kk == kc - 1),
                            )
                    for j in range(gsz):
                        n0 = n0_base + j
                        for m0 in range(mt):
                            ot = opool.tile([128, 512], mybir.dt.float32, tag="o",
                                            name=f"o{b}_{n0}_{m0}")
                            nc.vector.tensor_copy(ot[:], ps[j, m0][:])
                            nc.scalar.dma_start(
                                out=out[b, n0 * 128:(n0 + 1) * 128,
                                        m0 * 512:(m0 + 1) * 512],
                                in_=ot[:],
                            )
                    n0_base += gsz
    nc.compile()
    return nc


def _dequant_wt(qweight, qrange, qmin):
    # Matches reference: w = q * qrange + qmin per (row, group), fp32.
    q = np.asarray(qweight).astype(np.float32).reshape(N, NGROUP, GS)
    qr = np.asarray(qrange).astype(np.float32).reshape(N, NGROUP, 1)
    qm = np.asarray(qmin).astype(np.float32).reshape(N, NGROUP, 1)
    w = (q * qr + qm).reshape(N, K)
    return np.ascontiguousarray(w.T).astype(ml_dtypes.bfloat16)  # (K, N)


def _ensure_axon_hooks():
    """run_bass_kernel_spmd(trace=True) imports antenv.axon_hooks, which some
    images lack; provide a stub (and register the real NTFF hook if the boot
    package is present) so tracing degrades gracefully instead of crashing."""
    try:
        import antenv.axon_hooks  # noqa: F401
        return
    except ImportError:
        pass
    try:
        import sys
        import types

        import antenv

        mod = types.ModuleType("antenv.axon_hooks")
        mod._hook = None
        mod.set_axon_ntff_profile_hook = lambda h: setattr(mod, "_hook", h)
        mod.get_axon_ntff_profile_hook = lambda: mod._hook
        sys.modules["antenv.axon_hooks"] = mod
        antenv.axon_hooks = mod
        try:
            from trn_agent_boot.trn_boot import _ntff_profile_via_ctypes

            mod._hook = _ntff_profile_via_ctypes("/opt/axon/libaxon_pjrt.so")
        except Exception:
            pass
    except Exception:
        pass


def kernel(x, qweight, qrange, qmin):
    global LAST_RESULT
    _ensure_axon_hooks()
    from concourse.bass_utils import run_bass_kernel_spmd

    wt_host = _dequant_wt(qweight, qrange, qmin)
    xb = np.asarray(x).astype(ml_dtypes.bfloat16)  # (B, K, M)

    if "nc" not in _CACHE:
        _CACHE["nc"] = _build_nc()
    nc = _CACHE["nc"]

    in_maps = [
        {"wt": wt_host, "xs": np.ascontiguousarray(xb[c * BPC:(c + 1) * BPC])}
        for c in range(NCORES)
    ]
    LAST_RESULT = run_bass_kernel_spmd(nc, in_maps, core_ids=list(range(NCORES)))
    outs = [r["out"] for r in LAST_RESULT.results]
    return np.ascontiguousarray(np.concatenate(outs, axis=0)).astype(np.float32, copy=False)
